# revision 23
# baseline (speedup 1.0000x reference)
"""ARMSNorm (int8 fake-quant RMS norm) Trainium2 kernel, 8-way data parallel.

Layout: x (4,4096,2048) f32 -> rows 16384 x 2048; core c owns rows
[c*2048, (c+1)*2048). Per core, the 16 MiB shard stays resident in SBUF:

  phase A: 16 per-tile (1 MiB) DMAs alternating between the Sync and
           Scalar HWDGE rings (earlier first byte + better HBM overlap);
           per-row absmax (DVE reduce) pipelined per tile, last tile in
           two column halves to shorten the tail -> local max
           -> AllGather(8) -> global max on partition 0 -> full scalar
           chain on p0 -> one gpsimd partition_broadcast
  phase B: x_int = round(x*inv_s) as int16 on DVE (RNE conversion,
           matching jnp.round); integer row sums of x_int^2 split across
           ACT (Square+accum), GpSimd/Pool (TT+reduce), and DVE (last
           tile); std = round(sqrt(var)) via boundary table (is_gt +
           reduce), interleaved per 4-column group, groups 0-2 on Pool,
           tail groups on DVE; row ymax -> AllGather(8) -> scale_out
  phase C: q = round(x_int * k_row) as int16 (DVE 2x-rate 16-bit ops);
           y = q*scale_out (DVE at 2x + a few tiles on ACT); 2-tile
           output chunks alternating Sync/Scalar rings.

HBM traffic per core: 16 MiB in + 8 MiB out (bf16) -- every element read
once and written once.
"""

from contextlib import ExitStack

import numpy as np

import concourse.bacc as bacc
import concourse.bass as bass
import concourse.bass_isa as bass_isa
import concourse.mybir as mybir
import concourse.tile as tile
from concourse import bass_utils

N_CORES = 8
P = 128

_cache: dict = {}


def _emit(nc, tc, x_dram, y_dram, w0: float, rows_per_core: int, d: int,
          wt_dram=None):
    f32, i32 = mybir.dt.float32, mybir.dt.int32
    i16, bf16 = mybir.dt.int16, mybir.dt.bfloat16
    OP = mybir.AluOpType
    AX = mybir.AxisListType.X
    AF = mybir.ActivationFunctionType
    T = rows_per_core // P          # 128-row (1 MiB) tiles
    RG = [list(range(N_CORES))]
    x_ap = x_dram.ap()
    y_ap = y_dram.ap()

    # engine split for the integer squares (T == 16 fast path).
    # Pool cannot free-axis reduce, so squares are ACT (Square+accum, one op)
    # except the last tile on DVE to shorten the serial tail.
    DVE_SQ = {15} if T == 16 else set()
    # phase-C y-scale tiles done on ACT (rest on DVE at 2x rate)
    ACT_Y = {1, 4, 7, 10, 13, 15} if T == 16 else set()
    # stats batches: first batch interleaved under the squares, second is
    # the short serial tail after the last square
    if T == 16:
        GROUPS = [(0, 12), (12, 16)]
    else:
        GROUPS = [(t, t + 1) for t in range(T)]

    def collective_ag(dr, name, src_p0):
        """[1,1] value on partition 0 of src -> AllGather -> [1,N] SBUF row on
        partition 0 (single-packet DMA both ways)."""
        ag_in = dr.tile([1, 1], f32, name=f"{name}_in")
        ag_out = dr.tile([N_CORES, 1], f32, name=f"{name}_out",
                         addr_space="Shared")
        nc.sync.dma_start(ag_in[:], src_p0)
        nc.gpsimd.collective_compute("AllGather", OP.bypass, replica_groups=RG,
                                     ins=[ag_in[:]], outs=[ag_out[:]])
        return ag_out

    with (
        tc.tile_pool(name="st", bufs=1) as st,
        tc.tile_pool(name="m16p", bufs=1) as m16p,
        tc.tile_pool(name="pp", bufs=2, space="PSUM") as pp,
        tc.tile_pool(name="dram", bufs=1, space="DRAM") as dr,
    ):
        # ---- stats buffers
        rowmax = st.tile([P, T], f32, name="rowmax")
        sums = st.tile([P, T], f32, name="sums")
        m16 = [m16p.tile([P, d], i16, name=f"m16_{t}") for t in range(T)]

        with ExitStack() as xstack:
            xp = xstack.enter_context(tc.tile_pool(name="xp", bufs=1))

            wtmp = st.tile([P, 1], f32, name="wtmp")
            nc.gpsimd.memset(wtmp[:], 0.0)
            if wt_dram is not None:
                wb = st.tile([P, d], f32, name="wb")
                nc.gpsimd.dma_start(wb[:], wt_dram.ap().broadcast_to([P, d]))

            # ---- phase A: decreasing-size chunked loads split across the
            # Scalar ring (free of framework traffic -> first byte ~7us) and
            # the Sync ring, + per-chunk row absmax; the last tile is split
            # in column halves so its reduce overlaps the in-flight half.
            xw = [None] * T
            h = d // 2
            # (chunk_width, ring) lists; ~60/40 byte split between rings
            if T == 16:
                IN_CHUNKS = [(4, nc.scalar), (3, nc.sync), (3, nc.scalar),
                             (2, nc.sync), (2, nc.scalar), (1, nc.sync)]
            else:
                IN_CHUNKS = [(1, nc.scalar)] * (T - 1)
            s = 0
            first = True
            for w, ring in IN_CHUNKS:
                if w == 1:
                    xw[s] = xp.tile([P, d], f32, name=f"xw{s}")
                    ring.dma_start(xw[s][:], x_ap[s * P:(s + 1) * P, :])
                    nc.vector.tensor_reduce(out=rowmax[:, s:s + 1],
                                            in_=xw[s][:], axis=AX, op=OP.max,
                                            apply_absolute_value=True)
                else:
                    xc = xp.tile([P, w, d], f32, name=f"xc{s}")
                    ring.dma_start(
                        xc[:],
                        x_ap[s * P:(s + w) * P, :].rearrange(
                            "(f p) d -> p f d", p=P))
                    for j in range(w):
                        xw[s + j] = xc[:, j:j + 1, :].squeeze()
                    nc.vector.tensor_reduce(out=rowmax[:, s:s + w],
                                            in_=xc[:], axis=AX, op=OP.max,
                                            apply_absolute_value=True)
                s += w
                if first:
                    # prefetch BOTH ACT tables (Square: sel 0, Sqrt: sel 1)
                    # while the loads stream; emitted after the first chunk
                    # issue so they don't delay the first data packet
                    warm_act = st.tile([P, 1], f32, name="warm_act")
                    nc.scalar.activation(warm_act[:], wtmp[:], AF.Square,
                                         bias=0.0, scale=1.0)
                    warm_act2 = st.tile([P, 1], f32, name="warm_act2")
                    nc.scalar.activation(warm_act2[:], wtmp[:], AF.Sqrt,
                                         bias=0.0, scale=1.0)
                    first = False
            # last tile in two halves
            tl = T - 1
            xw[tl] = xp.tile([P, d], f32, name=f"xw{tl}")
            nc.scalar.dma_start(xw[tl][:, :h], x_ap[tl * P:(tl + 1) * P, :h])
            nc.scalar.dma_start(xw[tl][:, h:], x_ap[tl * P:(tl + 1) * P, h:])
            rm15 = st.tile([P, 2], f32, name="rm15")
            nc.vector.tensor_reduce(out=rm15[:, 0:1], in_=xw[tl][:, :h],
                                    axis=AX, op=OP.max,
                                    apply_absolute_value=True)
            nc.vector.tensor_reduce(out=rm15[:, 1:2], in_=xw[tl][:, h:],
                                    axis=AX, op=OP.max,
                                    apply_absolute_value=True)
            nc.vector.tensor_tensor(out=rowmax[:, tl:tl + 1], in0=rm15[:, 0:1],
                                    in1=rm15[:, 1:2], op=OP.max)

            # local max: bulk reduce early, fold the last two tiles at the end
            lmax_a = st.tile([P, 1], f32, name="lmax_a")
            nc.vector.tensor_reduce(out=lmax_a[:], in_=rowmax[:, :T - 2],
                                    axis=AX, op=OP.max)
            lmax_b = st.tile([P, 1], f32, name="lmax_b")
            nc.vector.tensor_tensor(out=lmax_b[:], in0=lmax_a[:],
                                    in1=rowmax[:, T - 2:T - 1], op=OP.max)
            lmax = st.tile([P, 1], f32, name="lmax")
            nc.vector.tensor_tensor(out=lmax[:], in0=lmax_b[:],
                                    in1=rowmax[:, T - 1:T], op=OP.max)
            pr1 = st.tile([P, 1], f32, name="pr1")
            nc.gpsimd.partition_all_reduce(pr1[:], lmax[:], channels=P,
                                           reduce_op=bass_isa.ReduceOp.max)
            ag1_out = collective_ag(dr, "ag1", pr1[:1, :])

            # ---- AR1 return: single-packet row DMA + p0 chain + broadcast
            gm_row = st.tile([1, N_CORES], f32, name="gm_row")
            nc.sync.dma_start(gm_row[:], ag1_out[:].rearrange("e one -> one e"))
            sc_p0 = st.tile([1, 4], f32, name="sc_p0")
            gmax0 = st.tile([1, 1], f32, name="gmax0")
            nc.vector.tensor_reduce(out=gmax0[:], in_=gm_row[:], axis=AX,
                                    op=OP.max)
            # cols: 0=scale_in 1=inv_s 2=sc2(=s^2/d) 3=siw_s(=s*|w0|)
            nc.vector.tensor_scalar(out=sc_p0[:, 0:1], in0=gmax0[:],
                                    scalar1=1.0 / 127.0, scalar2=1e-8,
                                    op0=OP.mult, op1=OP.max)
            nc.vector.reciprocal(sc_p0[:, 1:2], sc_p0[:, 0:1])
            nc.vector.tensor_scalar(out=sc_p0[:, 2:3], in0=sc_p0[:, 0:1],
                                    scalar1=sc_p0[:, 0:1], scalar2=1.0 / d,
                                    op0=OP.mult, op1=OP.mult)
            nc.vector.tensor_scalar(out=sc_p0[:, 3:4], in0=sc_p0[:, 0:1],
                                    scalar1=abs(w0), scalar2=None, op0=OP.mult)
            sc = st.tile([P, 4], f32, name="sc")
            nc.gpsimd.partition_broadcast(sc[:], sc_p0[:1, :], channels=P)
            scale_in, inv_s = sc[:, 0:1], sc[:, 1:2]
            sc2, siw_s = sc[:, 2:3], sc[:, 3:4]

            # ---- phase B: quantize (RNE, DVE) + integer square row sums
            var = st.tile([P, T], i32, name="var")
            stdf = st.tile([P, T], f32, name="stdf")
            stdi = st.tile([P, T], i32, name="stdi")
            stdr = st.tile([P, T], f32, name="stdr")
            sp1 = st.tile([P, T], f32, name="sp1")
            sm1 = st.tile([P, T], f32, name="sm1")
            bhi = st.tile([P, T], f32, name="bhi")
            blo = st.tile([P, T], f32, name="blo")
            gtc = st.tile([P, T], f32, name="gtc")
            lec = st.tile([P, T], f32, name="lec")
            tfx = st.tile([P, T], f32, name="tfx")
            stdx = st.tile([P, T], f32, name="stdx")
            inv_std = st.tile([P, T], f32, name="inv_std")
            rmx_i = st.tile([P, T], i32, name="rmx_i")
            if wt_dram is not None:
                wmax = st.tile([P, T], f32, name="wmax")
            ymr = st.tile([P, T], f32, name="ymr")

            def emit_square(t):
                if t in DVE_SQ:
                    sqv = st.tile([P, d], i16, name=f"sqv{t}", tag="sqv",
                                  bufs=2)
                    nc.vector.tensor_tensor(out=sqv[:], in0=m16[t][:],
                                            in1=m16[t][:], op=OP.mult)
                    nc.vector.tensor_reduce(out=sums[:, t:t + 1], in_=sqv[:],
                                            axis=AX, op=OP.add)
                else:
                    dump = pp.tile([P, d], f32, name=f"dump{t}", tag="dump")
                    nc.scalar.activation(dump[:], m16[t][:], AF.Square,
                                         bias=0.0, scale=1.0,
                                         accum_out=sums[:, t:t + 1])
                if wt_dram is not None:
                    mw_f = st.tile([P, d], f32, name=f"mw{t}", tag="mwf",
                                   bufs=2)
                    nc.vector.tensor_tensor(out=mw_f[:], in0=m16[t][:],
                                            in1=wb[:], op=OP.mult)
                    nc.vector.tensor_reduce(out=wmax[:, t:t + 1],
                                            in_=mw_f[:], axis=AX, op=OP.max,
                                            apply_absolute_value=True)

            def emit_stats(a, b):
                """var -> std = round(sqrt(var)) -> inv_std -> ymr for tile
                columns [a:b).  std comes from the ACT Sqrt table (same table
                set as Square -> no table reload) + RNE to int, made EXACT by
                a +-1 integer fixup against the q^2+q boundaries: round(
                sqrt(v)) = q iff q^2-q < v <= q^2+q for integer v."""
                cs = slice(a, b)
                ve = nc.vector
                ve.tensor_scalar(out=var[:, cs], in0=sums[:, cs],
                                 scalar1=sc2, scalar2=None, op0=OP.mult)
                nc.scalar.activation(stdf[:, cs], var[:, cs], AF.Sqrt,
                                     bias=0.0, scale=1.0)
                ve.tensor_scalar(out=stdi[:, cs], in0=stdf[:, cs],
                                 scalar1=1.0, scalar2=None, op0=OP.mult)
                ve.tensor_scalar(out=stdr[:, cs], in0=stdi[:, cs],
                                 scalar1=1.0, scalar2=None, op0=OP.mult)
                ve.tensor_scalar(out=sp1[:, cs], in0=stdr[:, cs],
                                 scalar1=1.0, scalar2=None, op0=OP.add)
                ve.tensor_scalar(out=sm1[:, cs], in0=stdr[:, cs],
                                 scalar1=-1.0, scalar2=None, op0=OP.add)
                ve.tensor_tensor(out=bhi[:, cs], in0=stdr[:, cs],
                                 in1=sp1[:, cs], op=OP.mult)
                ve.tensor_tensor(out=blo[:, cs], in0=stdr[:, cs],
                                 in1=sm1[:, cs], op=OP.mult)
                ve.tensor_tensor(out=gtc[:, cs], in0=var[:, cs],
                                 in1=bhi[:, cs], op=OP.is_gt)
                ve.tensor_tensor(out=lec[:, cs], in0=var[:, cs],
                                 in1=blo[:, cs], op=OP.is_le)
                ve.tensor_tensor(out=tfx[:, cs], in0=stdr[:, cs],
                                 in1=gtc[:, cs], op=OP.add)
                ve.tensor_tensor(out=stdx[:, cs], in0=tfx[:, cs],
                                 in1=lec[:, cs], op=OP.subtract)
                ve.reciprocal(inv_std[:, cs], stdx[:, cs])
                if wt_dram is None:
                    ve.tensor_tensor(out=ymr[:, cs], in0=rmx_i[:, cs],
                                     in1=inv_std[:, cs], op=OP.mult)
                else:
                    ve.tensor_tensor(out=ymr[:, cs], in0=wmax[:, cs],
                                     in1=inv_std[:, cs], op=OP.mult)

            gi = 0
            for t in range(T):
                nc.vector.tensor_scalar(out=m16[t][:], in0=xw[t][:],
                                        scalar1=inv_s, scalar2=None,
                                        op0=OP.mult)
                emit_square(t)
                if t == 2 and wt_dram is None:
                    # row |x_int| max = round(rowmax * inv_s), one batched op
                    nc.vector.tensor_scalar(out=rmx_i[:], in0=rowmax[:],
                                            scalar1=inv_s, scalar2=None,
                                            op0=OP.mult)
                while gi < len(GROUPS) and GROUPS[gi][1] == t + 1:
                    emit_stats(*GROUPS[gi])
                    gi += 1

        # x pool released here; phase-C pools reuse its SBUF space.
        with (
            tc.tile_pool(name="qp", bufs=4) as qp,
            tc.tile_pool(name="yp", bufs=1) as yp,
        ):
            ymax_l = st.tile([P, 1], f32, name="ymax_l")
            nc.vector.tensor_reduce(out=ymax_l[:], in_=ymr[:], axis=AX,
                                    op=OP.max)
            # fold the deferred s*|w0| factor into the row max (it commutes)
            ymax_s = st.tile([P, 1], f32, name="ymax_s")
            nc.vector.tensor_scalar(out=ymax_s[:], in0=ymax_l[:],
                                    scalar1=siw_s, scalar2=None, op0=OP.mult)
            pr2 = st.tile([P, 1], f32, name="pr2")
            nc.gpsimd.partition_all_reduce(pr2[:], ymax_s[:], channels=P,
                                           reduce_op=bass_isa.ReduceOp.max)
            ag2_out = collective_ag(dr, "ag2", pr2[:1, :])

            # ---- AR2 return: p0 chain + broadcast
            ym_row = st.tile([1, N_CORES], f32, name="ym_row")
            nc.sync.dma_start(ym_row[:], ag2_out[:].rearrange("e one -> one e"))
            so_p0 = st.tile([1, 2], f32, name="so_p0")
            ymax0 = st.tile([1, 1], f32, name="ymax0")
            nc.vector.tensor_reduce(out=ymax0[:], in_=ym_row[:], axis=AX,
                                    op=OP.max)
            # cols: 0=scale_out(clamped) 1=k0(=inv_so*scale_in*w0)
            nc.vector.tensor_scalar(out=so_p0[:, 0:1], in0=ymax0[:],
                                    scalar1=1.0 / 127.0, scalar2=1e-8,
                                    op0=OP.mult, op1=OP.max)
            inv_so0 = st.tile([1, 1], f32, name="inv_so0")
            nc.vector.reciprocal(inv_so0[:], so_p0[:, 0:1])
            nc.vector.tensor_scalar(out=so_p0[:, 1:2], in0=inv_so0[:],
                                    scalar1=sc_p0[:, 0:1], scalar2=float(w0),
                                    op0=OP.mult, op1=OP.mult)
            so = st.tile([P, 2], f32, name="so")
            nc.gpsimd.partition_broadcast(so[:], so_p0[:1, :], channels=P)
            so_b, k0 = so[:, 0:1], so[:, 1:2]
            k_row = st.tile([P, T], f32, name="k_row")
            nc.vector.tensor_scalar(out=k_row[:], in0=inv_std[:], scalar1=k0,
                                    scalar2=None, op0=OP.mult)

            # ---- phase C: requantize (RNE, DVE 2x) + y = q*scale_out;
            # 2-tile output chunks alternating rings (first two single-tile
            # for a fast DMA ramp)
            yt = yp.tile([P, T, d], bf16, name="yt")
            OUT_CHUNKS = [1] * T
            s = 0
            for ci, w in enumerate(OUT_CHUNKS):
                for j in range(w):
                    t = s + j
                    q_t = qp.tile([P, d], i16, name=f"q{t}", tag="q")
                    if wt_dram is None:
                        nc.vector.tensor_scalar(
                            out=q_t[:], in0=m16[t][:],
                            scalar1=k_row[:, t:t + 1], scalar2=None,
                            op0=OP.mult)
                    else:
                        mw_c = st.tile([P, d], f32, name=f"mwc{t}", tag="mwc",
                                       bufs=2)
                        nc.vector.tensor_tensor(out=mw_c[:], in0=m16[t][:],
                                                in1=wb[:], op=OP.mult)
                        nc.vector.tensor_scalar(
                            out=q_t[:], in0=mw_c[:],
                            scalar1=k_row[:, t:t + 1], scalar2=None,
                            op0=OP.mult)
                    ysl = yt[:, t:t + 1, :].squeeze()
                    if t in ACT_Y:
                        nc.scalar.activation(ysl, q_t[:], AF.Copy, bias=0.0,
                                             scale=so_b)
                    else:
                        nc.vector.tensor_scalar(out=ysl, in0=q_t[:],
                                                scalar1=so_b, scalar2=None,
                                                op0=OP.mult)
                nc.sync.dma_start(
                    y_ap[s * P:(s + w) * P, :].rearrange("(f p) d -> p f d",
                                                         p=P),
                    yt[:, s:s + w, :])
                s += w


def _build(w0, rows_per_core: int, d: int, uniform: bool = True):
    nc = bacc.Bacc("TRN2", target_bir_lowering=False, debug=False,
                   num_devices=N_CORES)
    x_dram = nc.dram_tensor("x", [rows_per_core, d], mybir.dt.float32,
                            kind="ExternalInput")
    wt_dram = None
    if not uniform:
        wt_dram = nc.dram_tensor("wt", [1, d], mybir.dt.float32,
                                 kind="ExternalInput")
    y_dram = nc.dram_tensor("y", [rows_per_core, d], mybir.dt.bfloat16,
                            kind="ExternalOutput")
    with tile.TileContext(nc) as tc:
        _emit(nc, tc, x_dram, y_dram,
              w0 if uniform else 1.0, rows_per_core, d, wt_dram=wt_dram)
    nc.compile()
    return nc


def kernel(x: np.ndarray, weight: np.ndarray, _trace: bool = False):
    x = np.asarray(x, dtype=np.float32)
    weight = np.asarray(weight, dtype=np.float32)
    rows = int(np.prod(x.shape[:-1]))
    d = x.shape[-1]
    rows_per_core = rows // N_CORES
    uniform = bool(np.all(weight == weight[0]))
    w0 = float(weight[0])

    key = (w0 if uniform else None, rows_per_core, d)
    if key not in _cache:
        _cache[key] = _build(w0, rows_per_core, d, uniform=uniform)
    nc = _cache[key]

    xf = np.ascontiguousarray(x.reshape(rows, d))
    in_maps = [
        {"x": xf[c * rows_per_core:(c + 1) * rows_per_core]}
        for c in range(N_CORES)
    ]
    if not uniform:
        wrow = np.ascontiguousarray(weight.reshape(1, d))
        for m in in_maps:
            m["wt"] = wrow
    res = bass_utils.run_bass_kernel_spmd(nc, in_maps,
                                          core_ids=list(range(N_CORES)),
                                          trace=_trace)
    y = np.concatenate([np.asarray(res.results[c]["y"], dtype=np.float32)
                        for c in range(N_CORES)], axis=0)
    out = y.reshape(x.shape)
    if _trace:
        return out, res
    return out


# revision 24
# speedup vs baseline: 1.0505x; 1.0505x over previous
"""ARMSNorm (int8 fake-quant RMS norm) Trainium2 kernel, 8-way data parallel.

Layout: x (4,4096,2048) f32 -> rows 16384 x 2048; core c owns rows
[c*2048, (c+1)*2048). Per core, the 16 MiB shard stays resident in SBUF:

  phase A: 16 per-tile (1 MiB) DMAs alternating between the Sync and
           Scalar HWDGE rings (earlier first byte + better HBM overlap);
           per-row absmax (DVE reduce) pipelined per tile, last tile in
           two column halves to shorten the tail -> local max
           -> AllGather(8) -> global max on partition 0 -> full scalar
           chain on p0 -> one gpsimd partition_broadcast
  phase B: x_int = round(x*inv_s) as int16 on DVE (RNE conversion,
           matching jnp.round); integer row sums of x_int^2 split across
           ACT (Square+accum), GpSimd/Pool (TT+reduce), and DVE (last
           tile); std = round(sqrt(var)) via boundary table (is_gt +
           reduce), interleaved per 4-column group, groups 0-2 on Pool,
           tail groups on DVE; row ymax -> AllGather(8) -> scale_out
  phase C: q = round(x_int * k_row) as int16 (DVE 2x-rate 16-bit ops);
           y = q*scale_out (DVE at 2x + a few tiles on ACT); 2-tile
           output chunks alternating Sync/Scalar rings.

HBM traffic per core: 16 MiB in + 8 MiB out (bf16) -- every element read
once and written once.
"""

from contextlib import ExitStack

import numpy as np

import concourse.bacc as bacc
import concourse.bass as bass
import concourse.bass_isa as bass_isa
import concourse.mybir as mybir
import concourse.tile as tile
from concourse import bass_utils

N_CORES = 8
P = 128

_cache: dict = {}


def _emit(nc, tc, x_dram, y_dram, w0: float, rows_per_core: int, d: int,
          wt_dram=None):
    f32, i32 = mybir.dt.float32, mybir.dt.int32
    i16, bf16 = mybir.dt.int16, mybir.dt.bfloat16
    OP = mybir.AluOpType
    AX = mybir.AxisListType.X
    AF = mybir.ActivationFunctionType
    T = rows_per_core // P          # 128-row (1 MiB) tiles
    RG = [list(range(N_CORES))]
    x_ap = x_dram.ap()
    y_ap = y_dram.ap()

    # engine split for the integer squares (T == 16 fast path).
    # Pool cannot free-axis reduce, so squares are ACT (Square+accum, one op)
    # except the last tile on DVE to shorten the serial tail.
    DVE_SQ = {15} if T == 16 else set()
    # phase-C y-scale tiles done on ACT (rest on DVE at 2x rate)
    ACT_Y = {1, 4, 7, 10, 13, 15} if T == 16 else set()
    # stats batches: first batch interleaved under the squares, second is
    # the short serial tail after the last square
    if T == 16:
        GROUPS = [(0, 12), (12, 16)]
    else:
        GROUPS = [(t, t + 1) for t in range(T)]

    def collective_ag(dr, name, src_p0):
        """[1,1] value on partition 0 of src -> AllGather -> [1,N] SBUF row on
        partition 0 (single-packet DMA both ways)."""
        ag_in = dr.tile([1, 1], f32, name=f"{name}_in")
        ag_out = dr.tile([N_CORES, 1], f32, name=f"{name}_out",
                         addr_space="Shared")
        nc.sync.dma_start(ag_in[:], src_p0)
        nc.gpsimd.collective_compute("AllGather", OP.bypass, replica_groups=RG,
                                     ins=[ag_in[:]], outs=[ag_out[:]])
        return ag_out

    with (
        tc.tile_pool(name="st", bufs=1) as st,
        tc.tile_pool(name="m16p", bufs=1) as m16p,
        tc.tile_pool(name="pp", bufs=2, space="PSUM") as pp,
        tc.tile_pool(name="dram", bufs=1, space="DRAM") as dr,
    ):
        # ---- stats buffers
        rowmax = st.tile([P, T], f32, name="rowmax")
        sums = st.tile([P, T], f32, name="sums")
        m16 = [m16p.tile([P, d], i16, name=f"m16_{t}") for t in range(T)]

        with ExitStack() as xstack:
            xp = xstack.enter_context(tc.tile_pool(name="xp", bufs=1))

            # ---- warmup collective (FULL replica group): a cross-core
            # barrier triggered ~13us in; absorbs per-core CC-stream/firmware
            # init skew while the input DMAs stream, so the real AllGathers
            # pay far less in their entry barrier.
            wtmp = st.tile([P, 1], f32, name="wtmp")
            nc.gpsimd.memset(wtmp[:], 0.0)
            warm_in = dr.tile([1, 1], f32, name="warm_in")
            warm_out = dr.tile([N_CORES, 1], f32, name="warm_out",
                               addr_space="Shared")
            nc.gpsimd.dma_start(warm_in[:], wtmp[:1, :])
            nc.gpsimd.collective_compute(
                "AllGather", OP.bypass, replica_groups=RG,
                ins=[warm_in[:]], outs=[warm_out[:]])
            if wt_dram is not None:
                wb = st.tile([P, d], f32, name="wb")
                nc.gpsimd.dma_start(wb[:], wt_dram.ap().broadcast_to([P, d]))

            # ---- phase A: decreasing-size chunked loads split across the
            # Scalar ring (free of framework traffic -> first byte ~7us) and
            # the Sync ring, + per-chunk row absmax; the last tile is split
            # in column halves so its reduce overlaps the in-flight half.
            xw = [None] * T
            h = d // 2
            # (chunk_width, ring) lists; ~60/40 byte split between rings
            if T == 16:
                IN_CHUNKS = [(4, nc.scalar), (3, nc.sync), (3, nc.scalar),
                             (2, nc.sync), (2, nc.scalar), (1, nc.sync)]
            else:
                IN_CHUNKS = [(1, nc.scalar)] * (T - 1)
            s = 0
            first = True
            for w, ring in IN_CHUNKS:
                if w == 1:
                    xw[s] = xp.tile([P, d], f32, name=f"xw{s}")
                    ring.dma_start(xw[s][:], x_ap[s * P:(s + 1) * P, :])
                    nc.vector.tensor_reduce(out=rowmax[:, s:s + 1],
                                            in_=xw[s][:], axis=AX, op=OP.max,
                                            apply_absolute_value=True)
                else:
                    xc = xp.tile([P, w, d], f32, name=f"xc{s}")
                    ring.dma_start(
                        xc[:],
                        x_ap[s * P:(s + w) * P, :].rearrange(
                            "(f p) d -> p f d", p=P))
                    for j in range(w):
                        xw[s + j] = xc[:, j:j + 1, :].squeeze()
                    nc.vector.tensor_reduce(out=rowmax[:, s:s + w],
                                            in_=xc[:], axis=AX, op=OP.max,
                                            apply_absolute_value=True)
                s += w
                if first:
                    # prefetch BOTH ACT tables (Square: sel 0, Sqrt: sel 1)
                    # while the loads stream; emitted after the first chunk
                    # issue so they don't delay the first data packet
                    warm_act = st.tile([P, 1], f32, name="warm_act")
                    nc.scalar.activation(warm_act[:], wtmp[:], AF.Square,
                                         bias=0.0, scale=1.0)
                    warm_act2 = st.tile([P, 1], f32, name="warm_act2")
                    nc.scalar.activation(warm_act2[:], wtmp[:], AF.Sqrt,
                                         bias=0.0, scale=1.0)
                    first = False
            # last tile in two halves
            tl = T - 1
            xw[tl] = xp.tile([P, d], f32, name=f"xw{tl}")
            nc.scalar.dma_start(xw[tl][:, :h], x_ap[tl * P:(tl + 1) * P, :h])
            nc.scalar.dma_start(xw[tl][:, h:], x_ap[tl * P:(tl + 1) * P, h:])
            rm15 = st.tile([P, 2], f32, name="rm15")
            nc.vector.tensor_reduce(out=rm15[:, 0:1], in_=xw[tl][:, :h],
                                    axis=AX, op=OP.max,
                                    apply_absolute_value=True)
            nc.vector.tensor_reduce(out=rm15[:, 1:2], in_=xw[tl][:, h:],
                                    axis=AX, op=OP.max,
                                    apply_absolute_value=True)
            nc.vector.tensor_tensor(out=rowmax[:, tl:tl + 1], in0=rm15[:, 0:1],
                                    in1=rm15[:, 1:2], op=OP.max)

            # local max: bulk reduce early, fold the last two tiles at the end
            lmax_a = st.tile([P, 1], f32, name="lmax_a")
            nc.vector.tensor_reduce(out=lmax_a[:], in_=rowmax[:, :T - 2],
                                    axis=AX, op=OP.max)
            lmax_b = st.tile([P, 1], f32, name="lmax_b")
            nc.vector.tensor_tensor(out=lmax_b[:], in0=lmax_a[:],
                                    in1=rowmax[:, T - 2:T - 1], op=OP.max)
            lmax = st.tile([P, 1], f32, name="lmax")
            nc.vector.tensor_tensor(out=lmax[:], in0=lmax_b[:],
                                    in1=rowmax[:, T - 1:T], op=OP.max)
            pr1 = st.tile([P, 1], f32, name="pr1")
            nc.gpsimd.partition_all_reduce(pr1[:], lmax[:], channels=P,
                                           reduce_op=bass_isa.ReduceOp.max)
            ag1_out = collective_ag(dr, "ag1", pr1[:1, :])

            # ---- AR1 return: single-packet row DMA + p0 chain + broadcast
            gm_row = st.tile([1, N_CORES], f32, name="gm_row")
            nc.sync.dma_start(gm_row[:], ag1_out[:].rearrange("e one -> one e"))
            sc_p0 = st.tile([1, 4], f32, name="sc_p0")
            gmax0 = st.tile([1, 1], f32, name="gmax0")
            nc.vector.tensor_reduce(out=gmax0[:], in_=gm_row[:], axis=AX,
                                    op=OP.max)
            # cols: 0=scale_in 1=inv_s 2=sc2(=s^2/d) 3=siw_s(=s*|w0|)
            nc.vector.tensor_scalar(out=sc_p0[:, 0:1], in0=gmax0[:],
                                    scalar1=1.0 / 127.0, scalar2=1e-8,
                                    op0=OP.mult, op1=OP.max)
            nc.vector.reciprocal(sc_p0[:, 1:2], sc_p0[:, 0:1])
            nc.vector.tensor_scalar(out=sc_p0[:, 2:3], in0=sc_p0[:, 0:1],
                                    scalar1=sc_p0[:, 0:1], scalar2=1.0 / d,
                                    op0=OP.mult, op1=OP.mult)
            nc.vector.tensor_scalar(out=sc_p0[:, 3:4], in0=sc_p0[:, 0:1],
                                    scalar1=abs(w0), scalar2=None, op0=OP.mult)
            sc = st.tile([P, 4], f32, name="sc")
            nc.gpsimd.partition_broadcast(sc[:], sc_p0[:1, :], channels=P)
            scale_in, inv_s = sc[:, 0:1], sc[:, 1:2]
            sc2, siw_s = sc[:, 2:3], sc[:, 3:4]

            # ---- phase B: quantize (RNE, DVE) + integer square row sums
            var = st.tile([P, T], i32, name="var")
            stdf = st.tile([P, T], f32, name="stdf")
            stdi = st.tile([P, T], i32, name="stdi")
            stdr = st.tile([P, T], f32, name="stdr")
            sp1 = st.tile([P, T], f32, name="sp1")
            sm1 = st.tile([P, T], f32, name="sm1")
            bhi = st.tile([P, T], f32, name="bhi")
            blo = st.tile([P, T], f32, name="blo")
            gtc = st.tile([P, T], f32, name="gtc")
            lec = st.tile([P, T], f32, name="lec")
            tfx = st.tile([P, T], f32, name="tfx")
            stdx = st.tile([P, T], f32, name="stdx")
            inv_std = st.tile([P, T], f32, name="inv_std")
            rmx_i = st.tile([P, T], i32, name="rmx_i")
            if wt_dram is not None:
                wmax = st.tile([P, T], f32, name="wmax")
            ymr = st.tile([P, T], f32, name="ymr")

            def emit_square(t):
                if t in DVE_SQ:
                    sqv = st.tile([P, d], i16, name=f"sqv{t}", tag="sqv",
                                  bufs=2)
                    nc.vector.tensor_tensor(out=sqv[:], in0=m16[t][:],
                                            in1=m16[t][:], op=OP.mult)
                    nc.vector.tensor_reduce(out=sums[:, t:t + 1], in_=sqv[:],
                                            axis=AX, op=OP.add)
                else:
                    dump = pp.tile([P, d], f32, name=f"dump{t}", tag="dump")
                    nc.scalar.activation(dump[:], m16[t][:], AF.Square,
                                         bias=0.0, scale=1.0,
                                         accum_out=sums[:, t:t + 1])
                if wt_dram is not None:
                    mw_f = st.tile([P, d], f32, name=f"mw{t}", tag="mwf",
                                   bufs=2)
                    nc.vector.tensor_tensor(out=mw_f[:], in0=m16[t][:],
                                            in1=wb[:], op=OP.mult)
                    nc.vector.tensor_reduce(out=wmax[:, t:t + 1],
                                            in_=mw_f[:], axis=AX, op=OP.max,
                                            apply_absolute_value=True)

            def emit_stats(a, b):
                """var -> std = round(sqrt(var)) -> inv_std -> ymr for tile
                columns [a:b).  std comes from the ACT Sqrt table (same table
                set as Square -> no table reload) + RNE to int, made EXACT by
                a +-1 integer fixup against the q^2+q boundaries: round(
                sqrt(v)) = q iff q^2-q < v <= q^2+q for integer v."""
                cs = slice(a, b)
                ve = nc.vector
                ve.tensor_scalar(out=var[:, cs], in0=sums[:, cs],
                                 scalar1=sc2, scalar2=None, op0=OP.mult)
                nc.scalar.activation(stdf[:, cs], var[:, cs], AF.Sqrt,
                                     bias=0.0, scale=1.0)
                ve.tensor_scalar(out=stdi[:, cs], in0=stdf[:, cs],
                                 scalar1=1.0, scalar2=None, op0=OP.mult)
                ve.tensor_scalar(out=stdr[:, cs], in0=stdi[:, cs],
                                 scalar1=1.0, scalar2=None, op0=OP.mult)
                ve.tensor_scalar(out=sp1[:, cs], in0=stdr[:, cs],
                                 scalar1=1.0, scalar2=None, op0=OP.add)
                ve.tensor_scalar(out=sm1[:, cs], in0=stdr[:, cs],
                                 scalar1=-1.0, scalar2=None, op0=OP.add)
                ve.tensor_tensor(out=bhi[:, cs], in0=stdr[:, cs],
                                 in1=sp1[:, cs], op=OP.mult)
                ve.tensor_tensor(out=blo[:, cs], in0=stdr[:, cs],
                                 in1=sm1[:, cs], op=OP.mult)
                ve.tensor_tensor(out=gtc[:, cs], in0=var[:, cs],
                                 in1=bhi[:, cs], op=OP.is_gt)
                ve.tensor_tensor(out=lec[:, cs], in0=var[:, cs],
                                 in1=blo[:, cs], op=OP.is_le)
                ve.tensor_tensor(out=tfx[:, cs], in0=stdr[:, cs],
                                 in1=gtc[:, cs], op=OP.add)
                ve.tensor_tensor(out=stdx[:, cs], in0=tfx[:, cs],
                                 in1=lec[:, cs], op=OP.subtract)
                ve.reciprocal(inv_std[:, cs], stdx[:, cs])
                if wt_dram is None:
                    ve.tensor_tensor(out=ymr[:, cs], in0=rmx_i[:, cs],
                                     in1=inv_std[:, cs], op=OP.mult)
                else:
                    ve.tensor_tensor(out=ymr[:, cs], in0=wmax[:, cs],
                                     in1=inv_std[:, cs], op=OP.mult)

            gi = 0
            for t in range(T):
                nc.vector.tensor_scalar(out=m16[t][:], in0=xw[t][:],
                                        scalar1=inv_s, scalar2=None,
                                        op0=OP.mult)
                emit_square(t)
                if t == 2 and wt_dram is None:
                    # row |x_int| max = round(rowmax * inv_s), one batched op
                    nc.vector.tensor_scalar(out=rmx_i[:], in0=rowmax[:],
                                            scalar1=inv_s, scalar2=None,
                                            op0=OP.mult)
                while gi < len(GROUPS) and GROUPS[gi][1] == t + 1:
                    emit_stats(*GROUPS[gi])
                    gi += 1

        # x pool released here; phase-C pools reuse its SBUF space.
        with (
            tc.tile_pool(name="qp", bufs=4) as qp,
            tc.tile_pool(name="yp", bufs=1) as yp,
        ):
            ymax_l = st.tile([P, 1], f32, name="ymax_l")
            nc.vector.tensor_reduce(out=ymax_l[:], in_=ymr[:], axis=AX,
                                    op=OP.max)
            # fold the deferred s*|w0| factor into the row max (it commutes)
            ymax_s = st.tile([P, 1], f32, name="ymax_s")
            nc.vector.tensor_scalar(out=ymax_s[:], in0=ymax_l[:],
                                    scalar1=siw_s, scalar2=None, op0=OP.mult)
            pr2 = st.tile([P, 1], f32, name="pr2")
            nc.gpsimd.partition_all_reduce(pr2[:], ymax_s[:], channels=P,
                                           reduce_op=bass_isa.ReduceOp.max)
            ag2_out = collective_ag(dr, "ag2", pr2[:1, :])

            # ---- AR2 return: p0 chain + broadcast
            ym_row = st.tile([1, N_CORES], f32, name="ym_row")
            nc.sync.dma_start(ym_row[:], ag2_out[:].rearrange("e one -> one e"))
            so_p0 = st.tile([1, 2], f32, name="so_p0")
            ymax0 = st.tile([1, 1], f32, name="ymax0")
            nc.vector.tensor_reduce(out=ymax0[:], in_=ym_row[:], axis=AX,
                                    op=OP.max)
            # cols: 0=scale_out(clamped) 1=k0(=inv_so*scale_in*w0)
            nc.vector.tensor_scalar(out=so_p0[:, 0:1], in0=ymax0[:],
                                    scalar1=1.0 / 127.0, scalar2=1e-8,
                                    op0=OP.mult, op1=OP.max)
            inv_so0 = st.tile([1, 1], f32, name="inv_so0")
            nc.vector.reciprocal(inv_so0[:], so_p0[:, 0:1])
            nc.vector.tensor_scalar(out=so_p0[:, 1:2], in0=inv_so0[:],
                                    scalar1=sc_p0[:, 0:1], scalar2=float(w0),
                                    op0=OP.mult, op1=OP.mult)
            so = st.tile([P, 2], f32, name="so")
            nc.gpsimd.partition_broadcast(so[:], so_p0[:1, :], channels=P)
            so_b, k0 = so[:, 0:1], so[:, 1:2]
            k_row = st.tile([P, T], f32, name="k_row")
            nc.vector.tensor_scalar(out=k_row[:], in0=inv_std[:], scalar1=k0,
                                    scalar2=None, op0=OP.mult)

            # ---- phase C: requantize (RNE, DVE 2x) + y = q*scale_out;
            # 2-tile output chunks alternating rings (first two single-tile
            # for a fast DMA ramp)
            yt = yp.tile([P, T, d], bf16, name="yt")
            OUT_CHUNKS = [1] * T
            s = 0
            for ci, w in enumerate(OUT_CHUNKS):
                for j in range(w):
                    t = s + j
                    q_t = qp.tile([P, d], i16, name=f"q{t}", tag="q")
                    if wt_dram is None:
                        nc.vector.tensor_scalar(
                            out=q_t[:], in0=m16[t][:],
                            scalar1=k_row[:, t:t + 1], scalar2=None,
                            op0=OP.mult)
                    else:
                        mw_c = st.tile([P, d], f32, name=f"mwc{t}", tag="mwc",
                                       bufs=2)
                        nc.vector.tensor_tensor(out=mw_c[:], in0=m16[t][:],
                                                in1=wb[:], op=OP.mult)
                        nc.vector.tensor_scalar(
                            out=q_t[:], in0=mw_c[:],
                            scalar1=k_row[:, t:t + 1], scalar2=None,
                            op0=OP.mult)
                    ysl = yt[:, t:t + 1, :].squeeze()
                    if t in ACT_Y:
                        nc.scalar.activation(ysl, q_t[:], AF.Copy, bias=0.0,
                                             scale=so_b)
                    else:
                        nc.vector.tensor_scalar(out=ysl, in0=q_t[:],
                                                scalar1=so_b, scalar2=None,
                                                op0=OP.mult)
                nc.sync.dma_start(
                    y_ap[s * P:(s + w) * P, :].rearrange("(f p) d -> p f d",
                                                         p=P),
                    yt[:, s:s + w, :])
                s += w


def _build(w0, rows_per_core: int, d: int, uniform: bool = True):
    nc = bacc.Bacc("TRN2", target_bir_lowering=False, debug=False,
                   num_devices=N_CORES)
    x_dram = nc.dram_tensor("x", [rows_per_core, d], mybir.dt.float32,
                            kind="ExternalInput")
    wt_dram = None
    if not uniform:
        wt_dram = nc.dram_tensor("wt", [1, d], mybir.dt.float32,
                                 kind="ExternalInput")
    y_dram = nc.dram_tensor("y", [rows_per_core, d], mybir.dt.bfloat16,
                            kind="ExternalOutput")
    with tile.TileContext(nc) as tc:
        _emit(nc, tc, x_dram, y_dram,
              w0 if uniform else 1.0, rows_per_core, d, wt_dram=wt_dram)
    nc.compile()
    return nc


def kernel(x: np.ndarray, weight: np.ndarray, _trace: bool = False):
    x = np.asarray(x, dtype=np.float32)
    weight = np.asarray(weight, dtype=np.float32)
    rows = int(np.prod(x.shape[:-1]))
    d = x.shape[-1]
    rows_per_core = rows // N_CORES
    uniform = bool(np.all(weight == weight[0]))
    w0 = float(weight[0])

    key = (w0 if uniform else None, rows_per_core, d)
    if key not in _cache:
        _cache[key] = _build(w0, rows_per_core, d, uniform=uniform)
    nc = _cache[key]

    xf = np.ascontiguousarray(x.reshape(rows, d))
    in_maps = [
        {"x": xf[c * rows_per_core:(c + 1) * rows_per_core]}
        for c in range(N_CORES)
    ]
    if not uniform:
        wrow = np.ascontiguousarray(weight.reshape(1, d))
        for m in in_maps:
            m["wt"] = wrow
    res = bass_utils.run_bass_kernel_spmd(nc, in_maps,
                                          core_ids=list(range(N_CORES)),
                                          trace=_trace)
    y = np.concatenate([np.asarray(res.results[c]["y"], dtype=np.float32)
                        for c in range(N_CORES)], axis=0)
    out = y.reshape(x.shape)
    if _trace:
        return out, res
    return out


# revision 25
# speedup vs baseline: 1.1079x; 1.0547x over previous
"""ARMSNorm (int8 fake-quant RMS norm) Trainium2 kernel, 8-way data parallel.

Layout: x (4,4096,2048) f32 -> rows 16384 x 2048; core c owns rows
[c*2048, (c+1)*2048). Per core, the 16 MiB shard stays resident in SBUF:

  phase A: 16 per-tile (1 MiB) DMAs alternating between the Sync and
           Scalar HWDGE rings (earlier first byte + better HBM overlap);
           per-row absmax (DVE reduce) pipelined per tile, last tile in
           two column halves to shorten the tail -> local max
           -> AllGather(8) -> global max on partition 0 -> full scalar
           chain on p0 -> one gpsimd partition_broadcast
  phase B: x_int = round(x*inv_s) as int16 on DVE (RNE conversion,
           matching jnp.round); integer row sums of x_int^2 split across
           ACT (Square+accum), GpSimd/Pool (TT+reduce), and DVE (last
           tile); std = round(sqrt(var)) via boundary table (is_gt +
           reduce), interleaved per 4-column group, groups 0-2 on Pool,
           tail groups on DVE; row ymax -> AllGather(8) -> scale_out
  phase C: q = round(x_int * k_row) as int16 (DVE 2x-rate 16-bit ops);
           y = q*scale_out (DVE at 2x + a few tiles on ACT); 2-tile
           output chunks alternating Sync/Scalar rings.

HBM traffic per core: 16 MiB in + 8 MiB out (bf16) -- every element read
once and written once.
"""

from contextlib import ExitStack

import numpy as np

import concourse.bacc as bacc
import concourse.bass as bass
import concourse.bass_isa as bass_isa
import concourse.mybir as mybir
import concourse.tile as tile
from concourse import bass_utils

N_CORES = 8
P = 128

_cache: dict = {}


def _emit(nc, tc, x_dram, y_dram, w0: float, rows_per_core: int, d: int,
          wt_dram=None):
    f32, i32 = mybir.dt.float32, mybir.dt.int32
    i16, bf16 = mybir.dt.int16, mybir.dt.bfloat16
    OP = mybir.AluOpType
    AX = mybir.AxisListType.X
    AF = mybir.ActivationFunctionType
    T = rows_per_core // P          # 128-row (1 MiB) tiles
    RG = [list(range(N_CORES))]
    x_ap = x_dram.ap()
    y_ap = y_dram.ap()

    # engine split for the integer squares (T == 16 fast path).
    # Pool cannot free-axis reduce, so squares are ACT (Square+accum, one op)
    # except the last tile on DVE to shorten the serial tail.
    DVE_SQ = {15} if T == 16 else set()
    # phase-C y-scale tiles done on ACT (rest on DVE at 2x rate)
    ACT_Y = {1, 4, 7, 10, 13, 15} if T == 16 else set()
    # stats batches: first batch interleaved under the squares, second is
    # the short serial tail after the last square
    if T == 16:
        GROUPS = [(0, 12), (12, 16)]
    else:
        GROUPS = [(t, t + 1) for t in range(T)]

    def collective_ag(dr, name, src_p0):
        """[1,1] value on partition 0 of src -> AllGather -> [1,N] SBUF row on
        partition 0 (single-packet DMA both ways)."""
        ag_in = dr.tile([1, 1], f32, name=f"{name}_in")
        ag_out = dr.tile([N_CORES, 1], f32, name=f"{name}_out",
                         addr_space="Shared")
        nc.sync.dma_start(ag_in[:], src_p0)
        nc.gpsimd.collective_compute("AllGather", OP.bypass, replica_groups=RG,
                                     ins=[ag_in[:]], outs=[ag_out[:]])
        return ag_out

    with (
        tc.tile_pool(name="st", bufs=1) as st,
        tc.tile_pool(name="m16p", bufs=1) as m16p,
        tc.tile_pool(name="pp", bufs=2, space="PSUM") as pp,
        tc.tile_pool(name="dram", bufs=1, space="DRAM") as dr,
    ):
        # ---- stats buffers
        rowmax = st.tile([P, T], f32, name="rowmax")
        sums = st.tile([P, T], f32, name="sums")
        m16 = [m16p.tile([P, d], i16, name=f"m16_{t}") for t in range(T)]

        with ExitStack() as xstack:
            xp = xstack.enter_context(tc.tile_pool(name="xp", bufs=1))

            # ---- warmup collective (FULL replica group): a cross-core
            # barrier triggered ~13us in; absorbs per-core CC-stream/firmware
            # init skew while the input DMAs stream, so the real AllGathers
            # pay far less in their entry barrier.
            wtmp = st.tile([P, 1], f32, name="wtmp")
            nc.gpsimd.memset(wtmp[:], 0.0)
            warm_in = dr.tile([1, 1], f32, name="warm_in")
            warm_out = dr.tile([N_CORES, 1], f32, name="warm_out",
                               addr_space="Shared")
            nc.gpsimd.dma_start(warm_in[:], wtmp[:1, :])
            nc.gpsimd.collective_compute(
                "AllGather", OP.bypass, replica_groups=RG,
                ins=[warm_in[:]], outs=[warm_out[:]])
            if wt_dram is not None:
                wb = st.tile([P, d], f32, name="wb")
                nc.gpsimd.dma_start(wb[:], wt_dram.ap().broadcast_to([P, d]))

            # ---- phase A: chunked loads on the Scalar ring (free of
            # framework traffic -> first byte ~7us) + per-chunk row absmax.
            # Small chunks first (fast pipeline fill) and last (short tail);
            # single ring so arrival order == emission order and the DVE
            # reduce queue never head-of-line blocks on a late chunk.
            # The last tile is split in column halves so its reduce overlaps
            # the in-flight second half.
            xw = [None] * T
            h = d // 2
            if T == 16:
                IN_CHUNKS = [(1, nc.scalar), (1, nc.scalar), (2, nc.scalar),
                             (2, nc.scalar), (2, nc.scalar), (2, nc.scalar),
                             (2, nc.scalar), (2, nc.scalar), (1, nc.scalar)]
            else:
                IN_CHUNKS = [(1, nc.scalar)] * (T - 1)
            s = 0
            first = True
            for w, ring in IN_CHUNKS:
                if w == 1:
                    xw[s] = xp.tile([P, d], f32, name=f"xw{s}")
                    ring.dma_start(xw[s][:], x_ap[s * P:(s + 1) * P, :])
                    nc.vector.tensor_reduce(out=rowmax[:, s:s + 1],
                                            in_=xw[s][:], axis=AX, op=OP.max,
                                            apply_absolute_value=True)
                else:
                    xc = xp.tile([P, w, d], f32, name=f"xc{s}")
                    ring.dma_start(
                        xc[:],
                        x_ap[s * P:(s + w) * P, :].rearrange(
                            "(f p) d -> p f d", p=P))
                    for j in range(w):
                        xw[s + j] = xc[:, j:j + 1, :].squeeze()
                    nc.vector.tensor_reduce(out=rowmax[:, s:s + w],
                                            in_=xc[:], axis=AX, op=OP.max,
                                            apply_absolute_value=True)
                s += w
                if first:
                    # prefetch BOTH ACT tables (Square: sel 0, Sqrt: sel 1)
                    # while the loads stream; emitted after the first chunk
                    # issue so they don't delay the first data packet
                    warm_act = st.tile([P, 1], f32, name="warm_act")
                    nc.scalar.activation(warm_act[:], wtmp[:], AF.Square,
                                         bias=0.0, scale=1.0)
                    warm_act2 = st.tile([P, 1], f32, name="warm_act2")
                    nc.scalar.activation(warm_act2[:], wtmp[:], AF.Sqrt,
                                         bias=0.0, scale=1.0)
                    first = False
            # last tile in two halves
            tl = T - 1
            xw[tl] = xp.tile([P, d], f32, name=f"xw{tl}")
            nc.scalar.dma_start(xw[tl][:, :h], x_ap[tl * P:(tl + 1) * P, :h])
            nc.scalar.dma_start(xw[tl][:, h:], x_ap[tl * P:(tl + 1) * P, h:])
            rm15 = st.tile([P, 2], f32, name="rm15")
            nc.vector.tensor_reduce(out=rm15[:, 0:1], in_=xw[tl][:, :h],
                                    axis=AX, op=OP.max,
                                    apply_absolute_value=True)
            nc.vector.tensor_reduce(out=rm15[:, 1:2], in_=xw[tl][:, h:],
                                    axis=AX, op=OP.max,
                                    apply_absolute_value=True)
            nc.vector.tensor_tensor(out=rowmax[:, tl:tl + 1], in0=rm15[:, 0:1],
                                    in1=rm15[:, 1:2], op=OP.max)

            # local max: bulk reduce early, fold the last two tiles at the end
            lmax_a = st.tile([P, 1], f32, name="lmax_a")
            nc.vector.tensor_reduce(out=lmax_a[:], in_=rowmax[:, :T - 2],
                                    axis=AX, op=OP.max)
            lmax_b = st.tile([P, 1], f32, name="lmax_b")
            nc.vector.tensor_tensor(out=lmax_b[:], in0=lmax_a[:],
                                    in1=rowmax[:, T - 2:T - 1], op=OP.max)
            lmax = st.tile([P, 1], f32, name="lmax")
            nc.vector.tensor_tensor(out=lmax[:], in0=lmax_b[:],
                                    in1=rowmax[:, T - 1:T], op=OP.max)
            pr1 = st.tile([P, 1], f32, name="pr1")
            nc.gpsimd.partition_all_reduce(pr1[:], lmax[:], channels=P,
                                           reduce_op=bass_isa.ReduceOp.max)
            ag1_out = collective_ag(dr, "ag1", pr1[:1, :])

            # ---- AR1 return: single-packet row DMA + p0 chain + broadcast
            gm_row = st.tile([1, N_CORES], f32, name="gm_row")
            nc.sync.dma_start(gm_row[:], ag1_out[:].rearrange("e one -> one e"))
            sc_p0 = st.tile([1, 4], f32, name="sc_p0")
            gmax0 = st.tile([1, 1], f32, name="gmax0")
            nc.vector.tensor_reduce(out=gmax0[:], in_=gm_row[:], axis=AX,
                                    op=OP.max)
            # cols: 0=scale_in 1=inv_s 2=sc2(=s^2/d) 3=siw_s(=s*|w0|)
            nc.vector.tensor_scalar(out=sc_p0[:, 0:1], in0=gmax0[:],
                                    scalar1=1.0 / 127.0, scalar2=1e-8,
                                    op0=OP.mult, op1=OP.max)
            nc.vector.reciprocal(sc_p0[:, 1:2], sc_p0[:, 0:1])
            nc.vector.tensor_scalar(out=sc_p0[:, 2:3], in0=sc_p0[:, 0:1],
                                    scalar1=sc_p0[:, 0:1], scalar2=1.0 / d,
                                    op0=OP.mult, op1=OP.mult)
            nc.vector.tensor_scalar(out=sc_p0[:, 3:4], in0=sc_p0[:, 0:1],
                                    scalar1=abs(w0), scalar2=None, op0=OP.mult)
            sc = st.tile([P, 4], f32, name="sc")
            nc.gpsimd.partition_broadcast(sc[:], sc_p0[:1, :], channels=P)
            scale_in, inv_s = sc[:, 0:1], sc[:, 1:2]
            sc2, siw_s = sc[:, 2:3], sc[:, 3:4]

            # ---- phase B: quantize (RNE, DVE) + integer square row sums
            var = st.tile([P, T], i32, name="var")
            stdf = st.tile([P, T], f32, name="stdf")
            stdi = st.tile([P, T], i32, name="stdi")
            stdr = st.tile([P, T], f32, name="stdr")
            sp1 = st.tile([P, T], f32, name="sp1")
            sm1 = st.tile([P, T], f32, name="sm1")
            bhi = st.tile([P, T], f32, name="bhi")
            blo = st.tile([P, T], f32, name="blo")
            gtc = st.tile([P, T], f32, name="gtc")
            lec = st.tile([P, T], f32, name="lec")
            tfx = st.tile([P, T], f32, name="tfx")
            stdx = st.tile([P, T], f32, name="stdx")
            inv_std = st.tile([P, T], f32, name="inv_std")
            rmx_i = st.tile([P, T], i32, name="rmx_i")
            if wt_dram is not None:
                wmax = st.tile([P, T], f32, name="wmax")
            ymr = st.tile([P, T], f32, name="ymr")

            def emit_square(t):
                if t in DVE_SQ:
                    sqv = st.tile([P, d], i16, name=f"sqv{t}", tag="sqv",
                                  bufs=2)
                    nc.vector.tensor_tensor(out=sqv[:], in0=m16[t][:],
                                            in1=m16[t][:], op=OP.mult)
                    nc.vector.tensor_reduce(out=sums[:, t:t + 1], in_=sqv[:],
                                            axis=AX, op=OP.add)
                else:
                    dump = pp.tile([P, d], f32, name=f"dump{t}", tag="dump")
                    nc.scalar.activation(dump[:], m16[t][:], AF.Square,
                                         bias=0.0, scale=1.0,
                                         accum_out=sums[:, t:t + 1])
                if wt_dram is not None:
                    mw_f = st.tile([P, d], f32, name=f"mw{t}", tag="mwf",
                                   bufs=2)
                    nc.vector.tensor_tensor(out=mw_f[:], in0=m16[t][:],
                                            in1=wb[:], op=OP.mult)
                    nc.vector.tensor_reduce(out=wmax[:, t:t + 1],
                                            in_=mw_f[:], axis=AX, op=OP.max,
                                            apply_absolute_value=True)

            def emit_stats(a, b):
                """var -> std = round(sqrt(var)) -> inv_std -> ymr for tile
                columns [a:b).  std comes from the ACT Sqrt table (same table
                set as Square -> no table reload) + RNE to int, made EXACT by
                a +-1 integer fixup against the q^2+q boundaries: round(
                sqrt(v)) = q iff q^2-q < v <= q^2+q for integer v."""
                cs = slice(a, b)
                ve = nc.vector
                ve.tensor_scalar(out=var[:, cs], in0=sums[:, cs],
                                 scalar1=sc2, scalar2=None, op0=OP.mult)
                nc.scalar.activation(stdf[:, cs], var[:, cs], AF.Sqrt,
                                     bias=0.0, scale=1.0)
                ve.tensor_scalar(out=stdi[:, cs], in0=stdf[:, cs],
                                 scalar1=1.0, scalar2=None, op0=OP.mult)
                ve.tensor_scalar(out=stdr[:, cs], in0=stdi[:, cs],
                                 scalar1=1.0, scalar2=None, op0=OP.mult)
                ve.tensor_scalar(out=sp1[:, cs], in0=stdr[:, cs],
                                 scalar1=1.0, scalar2=None, op0=OP.add)
                ve.tensor_scalar(out=sm1[:, cs], in0=stdr[:, cs],
                                 scalar1=-1.0, scalar2=None, op0=OP.add)
                ve.tensor_tensor(out=bhi[:, cs], in0=stdr[:, cs],
                                 in1=sp1[:, cs], op=OP.mult)
                ve.tensor_tensor(out=blo[:, cs], in0=stdr[:, cs],
                                 in1=sm1[:, cs], op=OP.mult)
                ve.tensor_tensor(out=gtc[:, cs], in0=var[:, cs],
                                 in1=bhi[:, cs], op=OP.is_gt)
                ve.tensor_tensor(out=lec[:, cs], in0=var[:, cs],
                                 in1=blo[:, cs], op=OP.is_le)
                ve.tensor_tensor(out=tfx[:, cs], in0=stdr[:, cs],
                                 in1=gtc[:, cs], op=OP.add)
                ve.tensor_tensor(out=stdx[:, cs], in0=tfx[:, cs],
                                 in1=lec[:, cs], op=OP.subtract)
                ve.reciprocal(inv_std[:, cs], stdx[:, cs])
                if wt_dram is None:
                    ve.tensor_tensor(out=ymr[:, cs], in0=rmx_i[:, cs],
                                     in1=inv_std[:, cs], op=OP.mult)
                else:
                    ve.tensor_tensor(out=ymr[:, cs], in0=wmax[:, cs],
                                     in1=inv_std[:, cs], op=OP.mult)

            gi = 0
            for t in range(T):
                nc.vector.tensor_scalar(out=m16[t][:], in0=xw[t][:],
                                        scalar1=inv_s, scalar2=None,
                                        op0=OP.mult)
                emit_square(t)
                if t == 2 and wt_dram is None:
                    # row |x_int| max = round(rowmax * inv_s), one batched op
                    nc.vector.tensor_scalar(out=rmx_i[:], in0=rowmax[:],
                                            scalar1=inv_s, scalar2=None,
                                            op0=OP.mult)
                while gi < len(GROUPS) and GROUPS[gi][1] == t + 1:
                    emit_stats(*GROUPS[gi])
                    gi += 1

        # x pool released here; phase-C pools reuse its SBUF space.
        with (
            tc.tile_pool(name="qp", bufs=4) as qp,
            tc.tile_pool(name="yp", bufs=1) as yp,
        ):
            ymax_l = st.tile([P, 1], f32, name="ymax_l")
            nc.vector.tensor_reduce(out=ymax_l[:], in_=ymr[:], axis=AX,
                                    op=OP.max)
            # fold the deferred s*|w0| factor into the row max (it commutes)
            ymax_s = st.tile([P, 1], f32, name="ymax_s")
            nc.vector.tensor_scalar(out=ymax_s[:], in0=ymax_l[:],
                                    scalar1=siw_s, scalar2=None, op0=OP.mult)
            pr2 = st.tile([P, 1], f32, name="pr2")
            nc.gpsimd.partition_all_reduce(pr2[:], ymax_s[:], channels=P,
                                           reduce_op=bass_isa.ReduceOp.max)
            ag2_out = collective_ag(dr, "ag2", pr2[:1, :])

            # ---- AR2 return: p0 chain + broadcast
            ym_row = st.tile([1, N_CORES], f32, name="ym_row")
            nc.sync.dma_start(ym_row[:], ag2_out[:].rearrange("e one -> one e"))
            so_p0 = st.tile([1, 2], f32, name="so_p0")
            ymax0 = st.tile([1, 1], f32, name="ymax0")
            nc.vector.tensor_reduce(out=ymax0[:], in_=ym_row[:], axis=AX,
                                    op=OP.max)
            # cols: 0=scale_out(clamped) 1=k0(=inv_so*scale_in*w0)
            nc.vector.tensor_scalar(out=so_p0[:, 0:1], in0=ymax0[:],
                                    scalar1=1.0 / 127.0, scalar2=1e-8,
                                    op0=OP.mult, op1=OP.max)
            inv_so0 = st.tile([1, 1], f32, name="inv_so0")
            nc.vector.reciprocal(inv_so0[:], so_p0[:, 0:1])
            nc.vector.tensor_scalar(out=so_p0[:, 1:2], in0=inv_so0[:],
                                    scalar1=sc_p0[:, 0:1], scalar2=float(w0),
                                    op0=OP.mult, op1=OP.mult)
            so = st.tile([P, 2], f32, name="so")
            nc.gpsimd.partition_broadcast(so[:], so_p0[:1, :], channels=P)
            so_b, k0 = so[:, 0:1], so[:, 1:2]
            k_row = st.tile([P, T], f32, name="k_row")
            nc.vector.tensor_scalar(out=k_row[:], in0=inv_std[:], scalar1=k0,
                                    scalar2=None, op0=OP.mult)

            # ---- phase C: requantize (RNE, DVE 2x) + y = q*scale_out;
            # 2-tile output chunks alternating rings (first two single-tile
            # for a fast DMA ramp)
            yt = yp.tile([P, T, d], bf16, name="yt")
            OUT_CHUNKS = [1] * T
            s = 0
            for ci, w in enumerate(OUT_CHUNKS):
                for j in range(w):
                    t = s + j
                    q_t = qp.tile([P, d], i16, name=f"q{t}", tag="q")
                    if wt_dram is None:
                        nc.vector.tensor_scalar(
                            out=q_t[:], in0=m16[t][:],
                            scalar1=k_row[:, t:t + 1], scalar2=None,
                            op0=OP.mult)
                    else:
                        mw_c = st.tile([P, d], f32, name=f"mwc{t}", tag="mwc",
                                       bufs=2)
                        nc.vector.tensor_tensor(out=mw_c[:], in0=m16[t][:],
                                                in1=wb[:], op=OP.mult)
                        nc.vector.tensor_scalar(
                            out=q_t[:], in0=mw_c[:],
                            scalar1=k_row[:, t:t + 1], scalar2=None,
                            op0=OP.mult)
                    ysl = yt[:, t:t + 1, :].squeeze()
                    if t in ACT_Y:
                        nc.scalar.activation(ysl, q_t[:], AF.Copy, bias=0.0,
                                             scale=so_b)
                    else:
                        nc.vector.tensor_scalar(out=ysl, in0=q_t[:],
                                                scalar1=so_b, scalar2=None,
                                                op0=OP.mult)
                nc.sync.dma_start(
                    y_ap[s * P:(s + w) * P, :].rearrange("(f p) d -> p f d",
                                                         p=P),
                    yt[:, s:s + w, :])
                s += w


def _build(w0, rows_per_core: int, d: int, uniform: bool = True):
    nc = bacc.Bacc("TRN2", target_bir_lowering=False, debug=False,
                   num_devices=N_CORES)
    x_dram = nc.dram_tensor("x", [rows_per_core, d], mybir.dt.float32,
                            kind="ExternalInput")
    wt_dram = None
    if not uniform:
        wt_dram = nc.dram_tensor("wt", [1, d], mybir.dt.float32,
                                 kind="ExternalInput")
    y_dram = nc.dram_tensor("y", [rows_per_core, d], mybir.dt.bfloat16,
                            kind="ExternalOutput")
    with tile.TileContext(nc) as tc:
        _emit(nc, tc, x_dram, y_dram,
              w0 if uniform else 1.0, rows_per_core, d, wt_dram=wt_dram)
    nc.compile()
    return nc


def kernel(x: np.ndarray, weight: np.ndarray, _trace: bool = False):
    x = np.asarray(x, dtype=np.float32)
    weight = np.asarray(weight, dtype=np.float32)
    rows = int(np.prod(x.shape[:-1]))
    d = x.shape[-1]
    rows_per_core = rows // N_CORES
    uniform = bool(np.all(weight == weight[0]))
    w0 = float(weight[0])

    key = (w0 if uniform else None, rows_per_core, d)
    if key not in _cache:
        _cache[key] = _build(w0, rows_per_core, d, uniform=uniform)
    nc = _cache[key]

    xf = np.ascontiguousarray(x.reshape(rows, d))
    in_maps = [
        {"x": xf[c * rows_per_core:(c + 1) * rows_per_core]}
        for c in range(N_CORES)
    ]
    if not uniform:
        wrow = np.ascontiguousarray(weight.reshape(1, d))
        for m in in_maps:
            m["wt"] = wrow
    res = bass_utils.run_bass_kernel_spmd(nc, in_maps,
                                          core_ids=list(range(N_CORES)),
                                          trace=_trace)
    y = np.concatenate([np.asarray(res.results[c]["y"], dtype=np.float32)
                        for c in range(N_CORES)], axis=0)
    out = y.reshape(x.shape)
    if _trace:
        return out, res
    return out
